# revision 28
# baseline (speedup 1.0000x reference)
"""Tensor-parallel Llama attention for 8 TRN2 NeuronCores.

Sharding: core d handles batch d//4 and q-head group g = d%4 (q heads
4g..4g+3, kv head g — GQA group-aligned so each core needs exactly one
kv head).  Wq/Wk/Wv are row-sharded, Wo column-sharded; the per-batch
partial o_proj outputs of 4 cores are summed on the host.

Device layouts (prepared host-side, bf16):
  hsT  [16,128,S]   hidden_states[b].T, HID on partitions in 16 chunks
  wqT  [16,128,512] Wq_shard.T          wkT/wvT [16,128,128]
  woT  [4,128,2048] Wo_shard.T (4 contraction chunks of the 512 local dims)
  cosT/sinT [128,S] RoPE tables in [head_dim, seq] layout
  mask [4,128,512]  0/1 causal masks for the 4 diagonal-block phases

Schedule (v2): K+V projections run contraction-chunk-outer with 8 PSUM
accumulators so the PE streams behind the initial hsT DMA without
stalling.  Q projection, attention, and o_proj are software-pipelined
per 512-query group gi: q(gi+2) is emitted between attention(gi) and
o_proj(gi) to hide the softmax-normalize latency.  Diagonal attention
blocks are width-trimmed to the causal triangle.  Softmax sums
accumulate directly into a packed PSUM tile (rows 32h); denominators
use reciprocal_approx_fast; pv*(1/sum) is fused into one
scalar_tensor_tensor.  o_proj PSUM->SBUF copies alternate DVE/ACT.
"""

import sys

sys.path.insert(0, "/opt/trn_rl_repo")

import numpy as np
import ml_dtypes

B, S, HID = 2, 2048, 2048
NH, NKV, HD = 16, 4, 128
THETA = 10000.0
NCORES = 8
HPC = 4            # q heads per core
QDIM = HPC * HD    # 512 local q dims
KT = HID // 128    # 16 contraction chunks
SB = S // 512      # 4 column groups of 512
ST = S // 128      # 16 row tiles of 128

_CACHE = {}


def _patch_tile_drain():
    """This walrus build caps sync waits per CTRL instruction below what the
    stock Tile kernel-tail drain carries; split them into single-wait NOPs."""
    import bass_rust
    import concourse.tile as tile
    from concourse.tile import ScopedClock

    if getattr(tile.TileContext, "_drain_split_patched", False):
        return

    def _split_drain_and_barrier(self, tick_clock, wait_clock):
        ticks = list(tick_clock.global_clock)
        for i, v in enumerate(ticks):
            if v > 0:
                single = [0] * len(ticks)
                single[i] = v
                nop = self.nc.sync.nop(nofuse=True, hint=f"drain_wait_{i}")
                wait_clock.add_sem_waits(
                    nop.ins, ScopedClock({None: bass_rust.VectorClock(single)})
                )
        self.nc.sync.drain()
        self.nc.all_engine_barrier()
        assert self.sems is not None
        popped = self.nc._tile_sem_poison_stack.pop()
        assert popped is self._sem_poison
        self.nc.clear_and_free_semaphores(list(self.sems.allocated().values()))
        self.nc.all_engine_barrier()

    tile.TileContext._drain_and_barrier = _split_drain_and_barrier
    tile.TileContext._drain_split_patched = True


def _legalize_waits(nc, max_waits=1):
    """This walrus build rejects instructions carrying more than ~2 sync
    waits.  Hoist the excess onto single-wait NOPs inserted just before the
    instruction in its block (same engine => same instruction stream, so
    the waits still complete before the op issues)."""
    import concourse.mybir as mybir

    n_split = 0
    for block in nc.m.functions[0].blocks:
        insts = list(block.instructions)
        out = []
        for inst in insts:
            si = getattr(inst, "sync_info", None)
            if si is not None and si.on_wait and len(si.on_wait) > max_waits:
                waits = list(si.on_wait)
                keep = waits[:max_waits]
                for j, w in enumerate(waits[max_waits:]):
                    out.append(
                        mybir.InstNoOp(
                            name=f"{inst.name}_hw{j}",
                            engine=inst.engine,
                            bass_nofuse=True,
                            sync_info=mybir.SyncInfo(on_wait=[w], on_update=[]),
                        )
                    )
                si.on_wait = keep
                n_split += 1
            out.append(inst)
        block.instructions = out
    return n_split


def _build_nc():
    import concourse.bass as bass
    import concourse.mybir as mybir
    import concourse.tile as tile
    from concourse.masks import make_identity

    _patch_tile_drain()

    bf = mybir.dt.bfloat16
    f32 = mybir.dt.float32
    Exp = mybir.ActivationFunctionType.Exp
    Ln = mybir.ActivationFunctionType.Ln
    Mult = mybir.AluOpType.mult

    nc = bass.Bass()
    hsT = nc.declare_dram_parameter("hsT", [KT, 128, S], bf, isOutput=False)
    wqT = nc.declare_dram_parameter("wqT", [128, KT * QDIM], bf, isOutput=False)
    wkT = nc.declare_dram_parameter("wkT", [128, KT * HD], bf, isOutput=False)
    wvT = nc.declare_dram_parameter("wvT", [128, KT * HD], bf, isOutput=False)
    woT = nc.declare_dram_parameter("woT", [128, 4 * HID], bf, isOutput=False)
    cosT = nc.declare_dram_parameter("cosT", [128, S], bf, isOutput=False)
    sinT = nc.declare_dram_parameter("sinT", [128, S], bf, isOutput=False)
    mask = nc.declare_dram_parameter("mask", [128, 4 * 512], bf, isOutput=False)
    out = nc.declare_dram_parameter("out", [S, HID], f32, isOutput=True)

    inv_sqrt_d = 1.0 / float(np.sqrt(HD))

    with tile.TileContext(nc) as tc:
        with (
            tc.tile_pool(name="resid", bufs=1) as resid,
            tc.tile_pool(name="probs", bufs=6) as probs_pool,
            tc.tile_pool(name="rot", bufs=2) as rot_pool,
            tc.tile_pool(name="rcp", bufs=2) as rcp_pool,
            tc.tile_pool(name="ostage", bufs=3) as ostage_pool,
        ):
            hs_sb = resid.tile([128, KT * S], bf)
            wq_sb = resid.tile([128, KT * QDIM], bf)
            wk_sb = resid.tile([128, KT * HD], bf)
            wv_sb = resid.tile([128, KT * HD], bf)
            wo_sb = resid.tile([128, 4 * HID], bf)
            cos_sb = resid.tile([128, S], bf)
            sin_sb = resid.tile([128, S], bf)
            mask_sb = resid.tile([128, 4 * 512], bf)
            ones_sb = resid.tile([128, 1], bf)
            ones128 = resid.tile([128, 128], bf)
            ident = resid.tile([128, 128], bf)
            qT_sb = resid.tile([128, HPC * S], bf)
            kT_sb = resid.tile([128, S], bf)
            vT_sb = resid.tile([128, S], bf)
            vn_sb = resid.tile([128, S], bf)
            at_sb = resid.tile([128, HPC * S], bf)

            # ---- loads: coalesced flat transfers, wq/cos/sin interleaved
            # into the hsT stream so nothing starves the projections ----
            nc.sync.dma_start(wk_sb[:], wkT[:])
            nc.sync.dma_start(wv_sb[:], wvT[:])
            QC = 4 * QDIM
            for kk in range(4):
                nc.sync.dma_start(hs_sb[:, kk * S:(kk + 1) * S], hsT[kk])
            nc.sync.dma_start(wq_sb[:, 0:QC], wqT[:, 0:QC])
            for kk in range(4, 8):
                nc.sync.dma_start(hs_sb[:, kk * S:(kk + 1) * S], hsT[kk])
            nc.sync.dma_start(cos_sb[:], cosT[:])
            nc.sync.dma_start(sin_sb[:], sinT[:])
            for kk in range(8, 12):
                nc.sync.dma_start(hs_sb[:, kk * S:(kk + 1) * S], hsT[kk])
            nc.sync.dma_start(wq_sb[:, QC:2 * QC], wqT[:, QC:2 * QC])
            for kk in range(12, KT):
                nc.sync.dma_start(hs_sb[:, kk * S:(kk + 1) * S], hsT[kk])
            nc.sync.dma_start(wq_sb[:, 2 * QC:3 * QC], wqT[:, 2 * QC:3 * QC])
            nc.sync.dma_start(wq_sb[:, 3 * QC:4 * QC], wqT[:, 3 * QC:4 * QC])
            nc.sync.dma_start(mask_sb[:], mask[:])
            nc.sync.dma_start(wo_sb[:], woT[:])
            nc.gpsimd.memset(ones_sb[:], 1.0)
            nc.gpsimd.memset(ones128[:], 1.0)
            make_identity(nc, ident[:])

            def rope_inplace(dst, ps, sg):
                # dst (sbuf bf16 [128,512]) = rope(ps [128,512]); sg picks
                # the cos/sin column group.
                cs = cos_sb[:, sg * 512:(sg + 1) * 512]
                sn = sin_sb[:, sg * 512:(sg + 1) * 512]
                rot = rot_pool.tile([128, 512], bf, name="rot", tag="rot")
                nc.vector.tensor_copy(dst, ps)
                nc.vector.tensor_scalar_mul(rot[0:64, :], dst[64:128, :], -1.0)
                nc.vector.tensor_copy(rot[64:128, :], dst[0:64, :])
                nc.vector.tensor_mul(dst, dst, cs)
                nc.vector.tensor_mul(rot[:], rot[:], sn)
                nc.vector.tensor_add(dst, dst, rot[:])

            # ---- k/v projections, contraction-chunk outer ----
            with tc.tile_pool(name="ldps", bufs=8, space="PSUM") as ldps:
                ldk = [ldps.tile([128, 512], f32, name=f"ldk{sg}", tag="ld")
                       for sg in range(SB)]
                ldv = [ldps.tile([128, 512], f32, name=f"ldv{sg}", tag="ld")
                       for sg in range(SB)]
                for kk in range(KT):
                    for sg in range(SB):
                        nc.tensor.matmul(
                            ldk[sg][:],
                            wk_sb[:, kk * HD: kk * HD + 128],
                            hs_sb[:, kk * S + sg * 512: kk * S + sg * 512 + 512],
                            start=(kk == 0), stop=(kk == KT - 1),
                        )
                        nc.tensor.matmul(
                            ldv[sg][:],
                            wv_sb[:, kk * HD: kk * HD + 128],
                            hs_sb[:, kk * S + sg * 512: kk * S + sg * 512 + 512],
                            start=(kk == 0), stop=(kk == KT - 1),
                        )
                # v copies on ACT first so the PE transposes can start while
                # the DVE is still busy with rope(k).
                for sg in range(SB):
                    nc.scalar.copy(vT_sb[:, sg * 512:(sg + 1) * 512], ldv[sg][:])
                for sg in range(SB):
                    rope_inplace(kT_sb[:, sg * 512:(sg + 1) * 512], ldk[sg][:], sg)

            with (
                tc.tile_pool(name="tr_ps", bufs=3, space="PSUM") as tr_ps,
                tc.tile_pool(name="pv_ps", bufs=4, space="PSUM") as pv_ps,
                tc.tile_pool(name="sm_ps", bufs=1, space="PSUM") as sm_ps,
            ):
                # ---- v back to natural [s, d] layout via PE transpose ----
                for tj in range(ST):
                    tp = tr_ps.tile([128, 128], bf, name="tp", tag="tr")
                    nc.tensor.transpose(tp[:], vT_sb[:, tj * 128:(tj + 1) * 128], ident[:])
                    nc.vector.tensor_copy(vn_sb[:, tj * 128:(tj + 1) * 128], tp[:])

                def qproj(gi):
                    for h in range(HPC):
                        ps = tr_ps.tile([128, 512], f32, name="qps", tag="tr")
                        for kk in range(KT):
                            nc.tensor.matmul(
                                ps[:],
                                wq_sb[:, kk * QDIM + h * 128: kk * QDIM + (h + 1) * 128],
                                hs_sb[:, kk * S + gi * 512: kk * S + gi * 512 + 512],
                                start=(kk == 0), stop=(kk == KT - 1),
                            )
                        rope_inplace(
                            qT_sb[:, h * S + gi * 512: h * S + gi * 512 + 512],
                            ps[:], gi,
                        )

                def attn(gi):
                    # Per-head tiles: off-diagonal j-tiles full width, the 4
                    # diagonal j-tiles width-trimmed to the causal triangle.
                    # tiles: list of (tj, i_off, width, mask_p)
                    tiles = []
                    for tj in range(4 * gi):
                        tiles.append((tj, 0, 512, None))
                    for p in range(4):
                        tiles.append((4 * gi + p, 128 * p, 512 - 128 * p, p))
                    ntile = len(tiles)

                    smp = sm_ps.tile([128, 512], f32, name="smp", tag="sm")
                    pvs = []

                    def emit_head(h):
                        qh = qT_sb[:, h * S + gi * 512: h * S + gi * 512 + 512]
                        pv = pv_ps.tile([128, 512], f32, name="pv", tag="pv")
                        pvs.append(pv)
                        # software pipeline with one-tile lookahead: emit
                        # score(t+1) before sum/pv(t) so exp overlaps PE.
                        pbs = [None] * ntile

                        def emit_score(t):
                            tj, ioff, w, p = tiles[t]
                            sc = tr_ps.tile([128, 512], f32, name="sc", tag="tr")
                            nc.tensor.matmul(
                                sc[:, 0:w],
                                kT_sb[:, tj * 128:(tj + 1) * 128],
                                qh[:, ioff:512],
                                start=True, stop=True,
                            )
                            pb = probs_pool.tile([128, 512], bf, name="pb")
                            nc.scalar.activation(
                                pb[:, 0:w], sc[:, 0:w], Exp, scale=inv_sqrt_d
                            )
                            if p is not None:
                                nc.vector.tensor_mul(
                                    pb[:, 0:w], pb[:, 0:w],
                                    mask_sb[:, p * 512 + 128 * p:(p + 1) * 512],
                                )
                            pbs[t] = pb

                        def emit_sumpv(t):
                            tj, ioff, w, p = tiles[t]
                            nc.tensor.matmul(
                                smp[32 * h:32 * h + 1, ioff:512],
                                ones_sb[:], pbs[t][:, 0:w],
                                start=(t == 0), stop=(t == ntile - 1),
                                skip_group_check=True,
                                tile_position=(0, 32 * h),
                            )
                            nc.tensor.matmul(
                                pv[:, ioff:512],
                                vn_sb[:, tj * 128:(tj + 1) * 128], pbs[t][:, 0:w],
                                start=(t == 0), stop=(t == ntile - 1),
                                skip_group_check=True,
                            )

                        emit_score(0)
                        if ntile > 1:
                            emit_score(1)
                        for t in range(ntile):
                            if t + 2 < ntile:
                                emit_score(t + 2)
                            emit_sumpv(t)

                    for h in range(HPC):
                        emit_head(h)
                    # batched 1/sums for all 4 heads (rows 32h of smp) as
                    # exp(-ln(s)) on ACT: ln/exp/copy share one act table, and
                    # this is ~2.6us cheaper than the DVE reciprocal.  The
                    # per-head broadcast+normalize follows in fin(gi), which
                    # the caller schedules behind PE filler work.
                    lns = rcp_pool.tile([128, 512], f32, name="lns", tag="rcp")
                    nc.scalar.activation(lns[:], smp[:], Ln)
                    rcpb = rcp_pool.tile([128, 512], bf, name="rcpb", tag="rcpb")
                    nc.scalar.activation(rcpb[:], lns[:], Exp, scale=-1.0)
                    return pvs, rcpb

                def fin(gi, pvs, rcpb):
                    for h in range(HPC):
                        bc = tr_ps.tile([128, 512], f32, name="bc", tag="tr")
                        nc.tensor.matmul(
                            bc[:], ones128[32 * h:32 * h + 1, :],
                            rcpb[32 * h:32 * h + 1, :],
                            start=True, stop=True,
                            tile_position=(32 * h, 0),
                        )
                        bcs = rcp_pool.tile([128, 512], bf, name="bcs", tag="bcs")
                        nc.scalar.copy(bcs[:], bc[:])
                        a_sl = at_sb[:, h * S + gi * 512: h * S + gi * 512 + 512]
                        nc.vector.scalar_tensor_tensor(
                            a_sl, pvs[h][:], 1.0, bcs[:], Mult, Mult
                        )

                def oproj(gi):
                    for st in range(4 * gi, 4 * gi + 4):
                        ostage = ostage_pool.tile([128, HID], f32, name="ostage")
                        for eg in range(SB):
                            ps = tr_ps.tile([128, 512], f32, name="ops", tag="tr")
                            for h in range(HPC):
                                nc.tensor.matmul(
                                    ps[:],
                                    at_sb[:, h * S + st * 128: h * S + st * 128 + 128],
                                    wo_sb[:, h * HID + eg * 512: h * HID + eg * 512 + 512],
                                    start=(h == 0), stop=(h == HPC - 1),
                                )
                            o_sl = ostage[:, eg * 512:(eg + 1) * 512]
                            if eg % 2 == 0:
                                nc.vector.tensor_copy(o_sl, ps[:])
                            else:
                                nc.scalar.copy(o_sl, ps[:])
                        nc.sync.dma_start(
                            out[st * 128:(st + 1) * 128, :], ostage[:]
                        )

                # Schedule: PE filler (qproj / delayed oproj) sits between
                # attn(gi) and fin(gi) so the batched reciprocal latency is
                # hidden; oproj(gi) runs one gi late for the same reason.
                qproj(0)
                qproj(1)
                f0 = attn(0)
                qproj(2)
                fin(0, *f0)
                f1 = attn(1)
                qproj(3)
                fin(1, *f1)
                f2 = attn(2)
                oproj(0)
                fin(2, *f2)
                f3 = attn(3)
                oproj(1)
                fin(3, *f3)
                oproj(2)
                oproj(3)
    _legalize_waits(nc)
    return nc


def _host_prep(hidden_states, Wq, Wk, Wv, Wo, position_ids):
    bf = ml_dtypes.bfloat16
    inv_freq = 1.0 / (THETA ** (np.arange(0, HD, 2, dtype=np.float64) / HD))

    mask = np.zeros((4, 128, 512), dtype=bf)
    jl = np.arange(128)[:, None]
    il = np.arange(512)[None, :]
    for p in range(4):
        mask[p] = (128 * p + jl <= il).astype(bf)
    # flat [128, 4*512] layout matching mask_sb
    mask_flat = np.ascontiguousarray(mask.transpose(1, 0, 2).reshape(128, 4 * 512))

    def flat(w):  # [KT,128,N] chunked -> [128, KT*N] sbuf layout
        return np.ascontiguousarray(w.transpose(1, 0, 2).reshape(128, -1))

    in_maps = []
    for d in range(NCORES):
        b, g = d // 4, d % 4
        hsT = np.ascontiguousarray(hidden_states[b].T).astype(bf).reshape(KT, 128, S)
        wqT = np.ascontiguousarray(Wq[g * QDIM:(g + 1) * QDIM].T).astype(bf).reshape(KT, 128, QDIM)
        wkT = np.ascontiguousarray(Wk[g * HD:(g + 1) * HD].T).astype(bf).reshape(KT, 128, HD)
        wvT = np.ascontiguousarray(Wv[g * HD:(g + 1) * HD].T).astype(bf).reshape(KT, 128, HD)
        woT = np.ascontiguousarray(Wo[:, g * QDIM:(g + 1) * QDIM].T).astype(bf).reshape(4, 128, HID)
        freqs = position_ids[b].astype(np.float64)[:, None] * inv_freq[None, :]  # [S, 64]
        emb = np.concatenate([freqs, freqs], axis=1)  # [S, 128]
        cosT = np.cos(emb).T.astype(bf)
        sinT = np.sin(emb).T.astype(bf)
        in_maps.append({
            "hsT": hsT, "wqT": flat(wqT), "wkT": flat(wkT), "wvT": flat(wvT),
            "woT": flat(woT),
            "cosT": np.ascontiguousarray(cosT),
            "sinT": np.ascontiguousarray(sinT),
            "mask": mask_flat,
        })
    return in_maps


def kernel(hidden_states, Wq, Wk, Wv, Wo, position_ids, _trace=False, _tmpdir=None):
    from concourse.bass_utils import run_bass_kernel_spmd

    if "nc" not in _CACHE:
        _CACHE["nc"] = _build_nc()
    nc = _CACHE["nc"]

    in_maps = _host_prep(
        np.asarray(hidden_states), np.asarray(Wq), np.asarray(Wk),
        np.asarray(Wv), np.asarray(Wo), np.asarray(position_ids),
    )
    res = run_bass_kernel_spmd(
        nc, in_maps, core_ids=list(range(NCORES)), trace=_trace, tmpdir=_tmpdir
    )
    _CACHE["last_result"] = res

    out = np.zeros((B, S, NH * HD), dtype=np.float32)
    for d in range(NCORES):
        out[d // 4] += res.results[d]["out"]
    return out


# revision 29
# speedup vs baseline: 1.0086x; 1.0086x over previous
"""Tensor-parallel Llama attention for 8 TRN2 NeuronCores.

Sharding: core d handles batch d//4 and q-head group g = d%4 (q heads
4g..4g+3, kv head g — GQA group-aligned so each core needs exactly one
kv head).  Wq/Wk/Wv are row-sharded, Wo column-sharded; the per-batch
partial o_proj outputs of 4 cores are summed on the host.

Device layouts (prepared host-side, bf16):
  hsT  [16,128,S]   hidden_states[b].T, HID on partitions in 16 chunks
  wqT  [16,128,512] Wq_shard.T          wkT/wvT [16,128,128]
  woT  [4,128,2048] Wo_shard.T (4 contraction chunks of the 512 local dims)
  cosT/sinT [128,S] RoPE tables in [head_dim, seq] layout
  mask [4,128,512]  0/1 causal masks for the 4 diagonal-block phases

Schedule (v2): K+V projections run contraction-chunk-outer with 8 PSUM
accumulators so the PE streams behind the initial hsT DMA without
stalling.  Q projection, attention, and o_proj are software-pipelined
per 512-query group gi: q(gi+2) is emitted between attention(gi) and
o_proj(gi) to hide the softmax-normalize latency.  Diagonal attention
blocks are width-trimmed to the causal triangle.  Softmax sums
accumulate directly into a packed PSUM tile (rows 32h); denominators
use reciprocal_approx_fast; pv*(1/sum) is fused into one
scalar_tensor_tensor.  o_proj PSUM->SBUF copies alternate DVE/ACT.
"""

import sys

sys.path.insert(0, "/opt/trn_rl_repo")

import numpy as np
import ml_dtypes

B, S, HID = 2, 2048, 2048
NH, NKV, HD = 16, 4, 128
THETA = 10000.0
NCORES = 8
HPC = 4            # q heads per core
QDIM = HPC * HD    # 512 local q dims
KT = HID // 128    # 16 contraction chunks
SB = S // 512      # 4 column groups of 512
ST = S // 128      # 16 row tiles of 128

_CACHE = {}


def _patch_tile_drain():
    """This walrus build caps sync waits per CTRL instruction below what the
    stock Tile kernel-tail drain carries; split them into single-wait NOPs."""
    import bass_rust
    import concourse.tile as tile
    from concourse.tile import ScopedClock

    if getattr(tile.TileContext, "_drain_split_patched", False):
        return

    def _split_drain_and_barrier(self, tick_clock, wait_clock):
        ticks = list(tick_clock.global_clock)
        for i, v in enumerate(ticks):
            if v > 0:
                single = [0] * len(ticks)
                single[i] = v
                nop = self.nc.sync.nop(nofuse=True, hint=f"drain_wait_{i}")
                wait_clock.add_sem_waits(
                    nop.ins, ScopedClock({None: bass_rust.VectorClock(single)})
                )
        self.nc.sync.drain()
        self.nc.all_engine_barrier()
        assert self.sems is not None
        popped = self.nc._tile_sem_poison_stack.pop()
        assert popped is self._sem_poison
        self.nc.clear_and_free_semaphores(list(self.sems.allocated().values()))
        self.nc.all_engine_barrier()

    tile.TileContext._drain_and_barrier = _split_drain_and_barrier
    tile.TileContext._drain_split_patched = True


def _legalize_waits(nc, max_waits=1):
    """This walrus build rejects instructions carrying more than ~2 sync
    waits.  Hoist the excess onto single-wait NOPs inserted just before the
    instruction in its block (same engine => same instruction stream, so
    the waits still complete before the op issues)."""
    import concourse.mybir as mybir

    n_split = 0
    for block in nc.m.functions[0].blocks:
        insts = list(block.instructions)
        out = []
        for inst in insts:
            si = getattr(inst, "sync_info", None)
            if si is not None and si.on_wait and len(si.on_wait) > max_waits:
                waits = list(si.on_wait)
                keep = waits[:max_waits]
                for j, w in enumerate(waits[max_waits:]):
                    out.append(
                        mybir.InstNoOp(
                            name=f"{inst.name}_hw{j}",
                            engine=inst.engine,
                            bass_nofuse=True,
                            sync_info=mybir.SyncInfo(on_wait=[w], on_update=[]),
                        )
                    )
                si.on_wait = keep
                n_split += 1
            out.append(inst)
        block.instructions = out
    return n_split


def _build_nc():
    import concourse.bass as bass
    import concourse.mybir as mybir
    import concourse.tile as tile
    from concourse.masks import make_identity

    _patch_tile_drain()

    bf = mybir.dt.bfloat16
    f32 = mybir.dt.float32
    Exp = mybir.ActivationFunctionType.Exp
    Ln = mybir.ActivationFunctionType.Ln
    Mult = mybir.AluOpType.mult

    nc = bass.Bass()
    hsT = nc.declare_dram_parameter("hsT", [KT, 128, S], bf, isOutput=False)
    wqT = nc.declare_dram_parameter("wqT", [128, KT * QDIM], bf, isOutput=False)
    wkT = nc.declare_dram_parameter("wkT", [128, KT * HD], bf, isOutput=False)
    wvT = nc.declare_dram_parameter("wvT", [128, KT * HD], bf, isOutput=False)
    woT = nc.declare_dram_parameter("woT", [128, 4 * HID], bf, isOutput=False)
    cosT = nc.declare_dram_parameter("cosT", [128, S], bf, isOutput=False)
    sinT = nc.declare_dram_parameter("sinT", [128, S], bf, isOutput=False)
    mask = nc.declare_dram_parameter("mask", [128, 4 * 512], bf, isOutput=False)
    out = nc.declare_dram_parameter("out", [S, HID], bf, isOutput=True)

    inv_sqrt_d = 1.0 / float(np.sqrt(HD))

    with tile.TileContext(nc) as tc:
        with (
            tc.tile_pool(name="resid", bufs=1) as resid,
            tc.tile_pool(name="probs", bufs=6) as probs_pool,
            tc.tile_pool(name="rot", bufs=2) as rot_pool,
            tc.tile_pool(name="rcp", bufs=2) as rcp_pool,
            tc.tile_pool(name="ostage", bufs=3) as ostage_pool,
        ):
            hs_sb = resid.tile([128, KT * S], bf)
            wq_sb = resid.tile([128, KT * QDIM], bf)
            wk_sb = resid.tile([128, KT * HD], bf)
            wv_sb = resid.tile([128, KT * HD], bf)
            wo_sb = resid.tile([128, 4 * HID], bf)
            cos_sb = resid.tile([128, S], bf)
            sin_sb = resid.tile([128, S], bf)
            mask_sb = resid.tile([128, 4 * 512], bf)
            ones_sb = resid.tile([128, 1], bf)
            ones128 = resid.tile([128, 128], bf)
            ident = resid.tile([128, 128], bf)
            qT_sb = resid.tile([128, HPC * S], bf)
            kT_sb = resid.tile([128, S], bf)
            vT_sb = resid.tile([128, S], bf)
            vn_sb = resid.tile([128, S], bf)
            at_sb = resid.tile([128, HPC * S], bf)

            # ---- loads: coalesced flat transfers, wq/cos/sin interleaved
            # into the hsT stream so nothing starves the projections ----
            nc.sync.dma_start(wk_sb[:], wkT[:])
            nc.sync.dma_start(wv_sb[:], wvT[:])
            QC = 4 * QDIM
            for kk in range(4):
                nc.sync.dma_start(hs_sb[:, kk * S:(kk + 1) * S], hsT[kk])
            nc.sync.dma_start(wq_sb[:, 0:QC], wqT[:, 0:QC])
            for kk in range(4, 8):
                nc.sync.dma_start(hs_sb[:, kk * S:(kk + 1) * S], hsT[kk])
            nc.sync.dma_start(cos_sb[:], cosT[:])
            nc.sync.dma_start(sin_sb[:], sinT[:])
            for kk in range(8, 12):
                nc.sync.dma_start(hs_sb[:, kk * S:(kk + 1) * S], hsT[kk])
            nc.sync.dma_start(wq_sb[:, QC:2 * QC], wqT[:, QC:2 * QC])
            for kk in range(12, KT):
                nc.sync.dma_start(hs_sb[:, kk * S:(kk + 1) * S], hsT[kk])
            nc.sync.dma_start(wq_sb[:, 2 * QC:3 * QC], wqT[:, 2 * QC:3 * QC])
            nc.sync.dma_start(wq_sb[:, 3 * QC:4 * QC], wqT[:, 3 * QC:4 * QC])
            nc.sync.dma_start(mask_sb[:], mask[:])
            nc.sync.dma_start(wo_sb[:], woT[:])
            nc.gpsimd.memset(ones_sb[:], 1.0)
            nc.gpsimd.memset(ones128[:], 1.0)
            make_identity(nc, ident[:])

            def rope_inplace(dst, ps, sg):
                # dst (sbuf bf16 [128,512]) = rope(ps [128,512]); sg picks
                # the cos/sin column group.
                cs = cos_sb[:, sg * 512:(sg + 1) * 512]
                sn = sin_sb[:, sg * 512:(sg + 1) * 512]
                rot = rot_pool.tile([128, 512], bf, name="rot", tag="rot")
                nc.vector.tensor_copy(dst, ps)
                nc.vector.tensor_scalar_mul(rot[0:64, :], dst[64:128, :], -1.0)
                nc.vector.tensor_copy(rot[64:128, :], dst[0:64, :])
                nc.vector.tensor_mul(dst, dst, cs)
                nc.vector.tensor_mul(rot[:], rot[:], sn)
                nc.vector.tensor_add(dst, dst, rot[:])

            # ---- k/v projections, contraction-chunk outer ----
            with tc.tile_pool(name="ldps", bufs=8, space="PSUM") as ldps:
                ldk = [ldps.tile([128, 512], f32, name=f"ldk{sg}", tag="ld")
                       for sg in range(SB)]
                ldv = [ldps.tile([128, 512], f32, name=f"ldv{sg}", tag="ld")
                       for sg in range(SB)]
                for kk in range(KT):
                    for sg in range(SB):
                        nc.tensor.matmul(
                            ldv[sg][:],
                            wv_sb[:, kk * HD: kk * HD + 128],
                            hs_sb[:, kk * S + sg * 512: kk * S + sg * 512 + 512],
                            start=(kk == 0), stop=(kk == KT - 1),
                        )
                        nc.tensor.matmul(
                            ldk[sg][:],
                            wk_sb[:, kk * HD: kk * HD + 128],
                            hs_sb[:, kk * S + sg * 512: kk * S + sg * 512 + 512],
                            start=(kk == 0), stop=(kk == KT - 1),
                        )
                # v copies on ACT first so the PE transposes can start while
                # the DVE is still busy with rope(k).
                for sg in range(SB):
                    nc.scalar.copy(vT_sb[:, sg * 512:(sg + 1) * 512], ldv[sg][:])
                for sg in range(SB):
                    rope_inplace(kT_sb[:, sg * 512:(sg + 1) * 512], ldk[sg][:], sg)

            with (
                tc.tile_pool(name="tr_ps", bufs=3, space="PSUM") as tr_ps,
                tc.tile_pool(name="pv_ps", bufs=4, space="PSUM") as pv_ps,
                tc.tile_pool(name="sm_ps", bufs=1, space="PSUM") as sm_ps,
            ):
                def vtrans():
                    # ---- v back to natural [s, d] layout via PE transpose ----
                    for tj in range(ST):
                        tp = tr_ps.tile([128, 128], bf, name="tp", tag="tr")
                        nc.tensor.transpose(tp[:], vT_sb[:, tj * 128:(tj + 1) * 128], ident[:])
                        nc.vector.tensor_copy(vn_sb[:, tj * 128:(tj + 1) * 128], tp[:])

                def qproj(gi):
                    for h in range(HPC):
                        ps = tr_ps.tile([128, 512], f32, name="qps", tag="tr")
                        for kk in range(KT):
                            nc.tensor.matmul(
                                ps[:],
                                wq_sb[:, kk * QDIM + h * 128: kk * QDIM + (h + 1) * 128],
                                hs_sb[:, kk * S + gi * 512: kk * S + gi * 512 + 512],
                                start=(kk == 0), stop=(kk == KT - 1),
                            )
                        rope_inplace(
                            qT_sb[:, h * S + gi * 512: h * S + gi * 512 + 512],
                            ps[:], gi,
                        )

                def attn(gi):
                    # Per-head tiles: off-diagonal j-tiles full width, the 4
                    # diagonal j-tiles width-trimmed to the causal triangle.
                    # tiles: list of (tj, i_off, width, mask_p)
                    tiles = []
                    for tj in range(4 * gi):
                        tiles.append((tj, 0, 512, None))
                    for p in range(4):
                        tiles.append((4 * gi + p, 128 * p, 512 - 128 * p, p))
                    ntile = len(tiles)

                    smp = sm_ps.tile([128, 512], f32, name="smp", tag="sm")
                    pvs = []

                    def emit_head(h):
                        qh = qT_sb[:, h * S + gi * 512: h * S + gi * 512 + 512]
                        pv = pv_ps.tile([128, 512], f32, name="pv", tag="pv")
                        pvs.append(pv)
                        # software pipeline with one-tile lookahead: emit
                        # score(t+1) before sum/pv(t) so exp overlaps PE.
                        pbs = [None] * ntile

                        def emit_score(t):
                            tj, ioff, w, p = tiles[t]
                            sc = tr_ps.tile([128, 512], f32, name="sc", tag="tr")
                            nc.tensor.matmul(
                                sc[:, 0:w],
                                kT_sb[:, tj * 128:(tj + 1) * 128],
                                qh[:, ioff:512],
                                start=True, stop=True,
                            )
                            pb = probs_pool.tile([128, 512], bf, name="pb")
                            nc.scalar.activation(
                                pb[:, 0:w], sc[:, 0:w], Exp, scale=inv_sqrt_d
                            )
                            if p is not None:
                                nc.vector.tensor_mul(
                                    pb[:, 0:w], pb[:, 0:w],
                                    mask_sb[:, p * 512 + 128 * p:(p + 1) * 512],
                                )
                            pbs[t] = pb

                        def emit_sumpv(t):
                            tj, ioff, w, p = tiles[t]
                            nc.tensor.matmul(
                                smp[32 * h:32 * h + 1, ioff:512],
                                ones_sb[:], pbs[t][:, 0:w],
                                start=(t == 0), stop=(t == ntile - 1),
                                skip_group_check=True,
                                tile_position=(0, 32 * h),
                            )
                            nc.tensor.matmul(
                                pv[:, ioff:512],
                                vn_sb[:, tj * 128:(tj + 1) * 128], pbs[t][:, 0:w],
                                start=(t == 0), stop=(t == ntile - 1),
                                skip_group_check=True,
                            )

                        emit_score(0)
                        if ntile > 1:
                            emit_score(1)
                        for t in range(ntile):
                            if t + 2 < ntile:
                                emit_score(t + 2)
                            emit_sumpv(t)

                    for h in range(HPC):
                        emit_head(h)
                    # batched 1/sums for all 4 heads (rows 32h of smp) as
                    # exp(-ln(s)) on ACT: ln/exp/copy share one act table, and
                    # this is ~2.6us cheaper than the DVE reciprocal.  The
                    # per-head broadcast+normalize follows in fin(gi), which
                    # the caller schedules behind PE filler work.
                    lns = rcp_pool.tile([128, 512], f32, name="lns", tag="rcp")
                    nc.scalar.activation(lns[:], smp[:], Ln)
                    rcpb = rcp_pool.tile([128, 512], bf, name="rcpb", tag="rcpb")
                    nc.scalar.activation(rcpb[:], lns[:], Exp, scale=-1.0)
                    return pvs, rcpb

                def fin(gi, pvs, rcpb):
                    for h in range(HPC):
                        bc = tr_ps.tile([128, 512], f32, name="bc", tag="tr")
                        nc.tensor.matmul(
                            bc[:], ones128[32 * h:32 * h + 1, :],
                            rcpb[32 * h:32 * h + 1, :],
                            start=True, stop=True,
                            tile_position=(32 * h, 0),
                        )
                        bcs = rcp_pool.tile([128, 512], bf, name="bcs", tag="bcs")
                        nc.scalar.copy(bcs[:], bc[:])
                        a_sl = at_sb[:, h * S + gi * 512: h * S + gi * 512 + 512]
                        nc.vector.scalar_tensor_tensor(
                            a_sl, pvs[h][:], 1.0, bcs[:], Mult, Mult
                        )

                def oproj(gi):
                    for st in range(4 * gi, 4 * gi + 4):
                        ostage = ostage_pool.tile([128, HID], bf, name="ostage")
                        for eg in range(SB):
                            ps = tr_ps.tile([128, 512], f32, name="ops", tag="tr")
                            for h in range(HPC):
                                nc.tensor.matmul(
                                    ps[:],
                                    at_sb[:, h * S + st * 128: h * S + st * 128 + 128],
                                    wo_sb[:, h * HID + eg * 512: h * HID + eg * 512 + 512],
                                    start=(h == 0), stop=(h == HPC - 1),
                                )
                            o_sl = ostage[:, eg * 512:(eg + 1) * 512]
                            if eg % 2 == 0:
                                nc.vector.tensor_copy(o_sl, ps[:])
                            else:
                                nc.scalar.copy(o_sl, ps[:])
                        nc.sync.dma_start(
                            out[st * 128:(st + 1) * 128, :], ostage[:]
                        )

                # Schedule: PE filler (qproj / delayed oproj) sits between
                # attn(gi) and fin(gi) so the batched reciprocal latency is
                # hidden; oproj(gi) runs one gi late for the same reason.
                qproj(0)
                vtrans()
                qproj(1)
                f0 = attn(0)
                qproj(2)
                fin(0, *f0)
                f1 = attn(1)
                qproj(3)
                fin(1, *f1)
                f2 = attn(2)
                oproj(0)
                fin(2, *f2)
                f3 = attn(3)
                oproj(1)
                fin(3, *f3)
                oproj(2)
                oproj(3)
    _legalize_waits(nc)
    return nc


def _host_prep(hidden_states, Wq, Wk, Wv, Wo, position_ids):
    bf = ml_dtypes.bfloat16
    inv_freq = 1.0 / (THETA ** (np.arange(0, HD, 2, dtype=np.float64) / HD))

    mask = np.zeros((4, 128, 512), dtype=bf)
    jl = np.arange(128)[:, None]
    il = np.arange(512)[None, :]
    for p in range(4):
        mask[p] = (128 * p + jl <= il).astype(bf)
    # flat [128, 4*512] layout matching mask_sb
    mask_flat = np.ascontiguousarray(mask.transpose(1, 0, 2).reshape(128, 4 * 512))

    def flat(w):  # [KT,128,N] chunked -> [128, KT*N] sbuf layout
        return np.ascontiguousarray(w.transpose(1, 0, 2).reshape(128, -1))

    in_maps = []
    for d in range(NCORES):
        b, g = d // 4, d % 4
        hsT = np.ascontiguousarray(hidden_states[b].T).astype(bf).reshape(KT, 128, S)
        wqT = np.ascontiguousarray(Wq[g * QDIM:(g + 1) * QDIM].T).astype(bf).reshape(KT, 128, QDIM)
        wkT = np.ascontiguousarray(Wk[g * HD:(g + 1) * HD].T).astype(bf).reshape(KT, 128, HD)
        wvT = np.ascontiguousarray(Wv[g * HD:(g + 1) * HD].T).astype(bf).reshape(KT, 128, HD)
        woT = np.ascontiguousarray(Wo[:, g * QDIM:(g + 1) * QDIM].T).astype(bf).reshape(4, 128, HID)
        freqs = position_ids[b].astype(np.float64)[:, None] * inv_freq[None, :]  # [S, 64]
        emb = np.concatenate([freqs, freqs], axis=1)  # [S, 128]
        cosT = np.cos(emb).T.astype(bf)
        sinT = np.sin(emb).T.astype(bf)
        in_maps.append({
            "hsT": hsT, "wqT": flat(wqT), "wkT": flat(wkT), "wvT": flat(wvT),
            "woT": flat(woT),
            "cosT": np.ascontiguousarray(cosT),
            "sinT": np.ascontiguousarray(sinT),
            "mask": mask_flat,
        })
    return in_maps


def kernel(hidden_states, Wq, Wk, Wv, Wo, position_ids, _trace=False, _tmpdir=None):
    from concourse.bass_utils import run_bass_kernel_spmd

    if "nc" not in _CACHE:
        _CACHE["nc"] = _build_nc()
    nc = _CACHE["nc"]

    in_maps = _host_prep(
        np.asarray(hidden_states), np.asarray(Wq), np.asarray(Wk),
        np.asarray(Wv), np.asarray(Wo), np.asarray(position_ids),
    )
    res = run_bass_kernel_spmd(
        nc, in_maps, core_ids=list(range(NCORES)), trace=_trace, tmpdir=_tmpdir
    )
    _CACHE["last_result"] = res

    out = np.zeros((B, S, NH * HD), dtype=np.float32)
    for d in range(NCORES):
        out[d // 4] += res.results[d]["out"].astype(np.float32)
    return out


# revision 30
# speedup vs baseline: 1.0349x; 1.0260x over previous
"""Tensor-parallel Llama attention for 8 TRN2 NeuronCores.

Sharding: core d handles batch d//4 and q-head group g = d%4 (q heads
4g..4g+3, kv head g — GQA group-aligned so each core needs exactly one
kv head).  Wq/Wk/Wv are row-sharded, Wo column-sharded; the per-batch
partial o_proj outputs of 4 cores are summed on the host.

Device layouts (prepared host-side, bf16):
  hsT  [16,128,S]   hidden_states[b].T, HID on partitions in 16 chunks
  wqT  [16,128,512] Wq_shard.T          wkT/wvT [16,128,128]
  woT  [4,128,2048] Wo_shard.T (4 contraction chunks of the 512 local dims)
  cosT/sinT [128,S] RoPE tables in [head_dim, seq] layout
  mask [4,128,512]  0/1 causal masks for the 4 diagonal-block phases

Schedule (v2): K+V projections run contraction-chunk-outer with 8 PSUM
accumulators so the PE streams behind the initial hsT DMA without
stalling.  Q projection, attention, and o_proj are software-pipelined
per 512-query group gi: q(gi+2) is emitted between attention(gi) and
o_proj(gi) to hide the softmax-normalize latency.  Diagonal attention
blocks are width-trimmed to the causal triangle.  Softmax sums
accumulate directly into a packed PSUM tile (rows 32h); denominators
use reciprocal_approx_fast; pv*(1/sum) is fused into one
scalar_tensor_tensor.  o_proj PSUM->SBUF copies alternate DVE/ACT.
"""

import sys

sys.path.insert(0, "/opt/trn_rl_repo")

import numpy as np
import ml_dtypes

B, S, HID = 2, 2048, 2048
NH, NKV, HD = 16, 4, 128
THETA = 10000.0
NCORES = 8
HPC = 4            # q heads per core
QDIM = HPC * HD    # 512 local q dims
KT = HID // 128    # 16 contraction chunks
SB = S // 512      # 4 column groups of 512
ST = S // 128      # 16 row tiles of 128

_CACHE = {}


def _patch_tile_drain():
    """This walrus build caps sync waits per CTRL instruction below what the
    stock Tile kernel-tail drain carries; split them into single-wait NOPs."""
    import bass_rust
    import concourse.tile as tile
    from concourse.tile import ScopedClock

    if getattr(tile.TileContext, "_drain_split_patched", False):
        return

    def _split_drain_and_barrier(self, tick_clock, wait_clock):
        ticks = list(tick_clock.global_clock)
        for i, v in enumerate(ticks):
            if v > 0:
                single = [0] * len(ticks)
                single[i] = v
                nop = self.nc.sync.nop(nofuse=True, hint=f"drain_wait_{i}")
                wait_clock.add_sem_waits(
                    nop.ins, ScopedClock({None: bass_rust.VectorClock(single)})
                )
        self.nc.sync.drain()
        self.nc.all_engine_barrier()
        assert self.sems is not None
        popped = self.nc._tile_sem_poison_stack.pop()
        assert popped is self._sem_poison
        self.nc.clear_and_free_semaphores(list(self.sems.allocated().values()))
        self.nc.all_engine_barrier()

    tile.TileContext._drain_and_barrier = _split_drain_and_barrier
    tile.TileContext._drain_split_patched = True


def _legalize_waits(nc, max_waits=1):
    """This walrus build rejects instructions carrying more than ~2 sync
    waits.  Hoist the excess onto single-wait NOPs inserted just before the
    instruction in its block (same engine => same instruction stream, so
    the waits still complete before the op issues)."""
    import concourse.mybir as mybir

    n_split = 0
    for block in nc.m.functions[0].blocks:
        insts = list(block.instructions)
        out = []
        for inst in insts:
            si = getattr(inst, "sync_info", None)
            if si is not None and si.on_wait and len(si.on_wait) > max_waits:
                waits = list(si.on_wait)
                keep = waits[:max_waits]
                for j, w in enumerate(waits[max_waits:]):
                    out.append(
                        mybir.InstNoOp(
                            name=f"{inst.name}_hw{j}",
                            engine=inst.engine,
                            bass_nofuse=True,
                            sync_info=mybir.SyncInfo(on_wait=[w], on_update=[]),
                        )
                    )
                si.on_wait = keep
                n_split += 1
            out.append(inst)
        block.instructions = out
    return n_split


def _build_nc():
    import concourse.bass as bass
    import concourse.mybir as mybir
    import concourse.tile as tile
    from concourse.masks import make_identity

    _patch_tile_drain()

    bf = mybir.dt.bfloat16
    f32 = mybir.dt.float32
    Exp = mybir.ActivationFunctionType.Exp
    Ln = mybir.ActivationFunctionType.Ln
    Mult = mybir.AluOpType.mult

    nc = bass.Bass()
    hsT = nc.declare_dram_parameter("hsT", [KT, 128, S], bf, isOutput=False)
    wqT = nc.declare_dram_parameter("wqT", [128, KT * QDIM], bf, isOutput=False)
    wkT = nc.declare_dram_parameter("wkT", [128, KT * HD], bf, isOutput=False)
    wvT = nc.declare_dram_parameter("wvT", [128, KT * HD], bf, isOutput=False)
    woT = nc.declare_dram_parameter("woT", [128, 4 * HID], bf, isOutput=False)
    cosT = nc.declare_dram_parameter("cosT", [128, S], bf, isOutput=False)
    sinT = nc.declare_dram_parameter("sinT", [128, S], bf, isOutput=False)
    mask = nc.declare_dram_parameter("mask", [128, 4 * 512], bf, isOutput=False)
    out = nc.declare_dram_parameter("out", [S, HID], bf, isOutput=True)

    inv_sqrt_d = 1.0 / float(np.sqrt(HD))

    with tile.TileContext(nc) as tc:
        with (
            tc.tile_pool(name="resid", bufs=1) as resid,
            tc.tile_pool(name="probs", bufs=6) as probs_pool,
            tc.tile_pool(name="rot", bufs=2) as rot_pool,
            tc.tile_pool(name="rcp", bufs=2) as rcp_pool,
            tc.tile_pool(name="ostage", bufs=3) as ostage_pool,
        ):
            hs_sb = resid.tile([128, KT * S], bf)
            wq_sb = resid.tile([128, KT * QDIM], bf)
            wk_sb = resid.tile([128, KT * HD], bf)
            wv_sb = resid.tile([128, KT * HD], bf)
            wo_sb = resid.tile([128, 4 * HID], bf)
            cos_sb = resid.tile([128, S], bf)
            sin_sb = resid.tile([128, S], bf)
            mask_sb = resid.tile([128, 4 * 512], bf)
            ones_sb = resid.tile([128, 1], bf)
            ones128 = resid.tile([128, 128], bf)
            ident = resid.tile([128, 128], bf)
            qT_sb = resid.tile([128, HPC * S], bf)
            kT_sb = resid.tile([128, S], bf)
            vT_sb = resid.tile([128, S], bf)
            vn_sb = resid.tile([128, S], bf)
            at_sb = resid.tile([128, HPC * S], bf)

            # ---- loads: coalesced flat transfers.  hsT streams first and
            # uninterrupted (it gates the k/v projections); wq follows —
            # q-proj consumes it far slower than DMA delivers ----
            nc.sync.dma_start(wk_sb[:], wkT[:])
            nc.sync.dma_start(wv_sb[:], wvT[:])
            QC = 4 * QDIM
            for kk in range(KT):
                nc.sync.dma_start(hs_sb[:, kk * S:(kk + 1) * S], hsT[kk])
            for c in range(4):
                nc.sync.dma_start(wq_sb[:, c * QC:(c + 1) * QC], wqT[:, c * QC:(c + 1) * QC])
            nc.sync.dma_start(cos_sb[:], cosT[:])
            nc.sync.dma_start(sin_sb[:], sinT[:])
            nc.sync.dma_start(mask_sb[:], mask[:])
            nc.sync.dma_start(wo_sb[:], woT[:])
            nc.gpsimd.memset(ones_sb[:], 1.0)
            nc.gpsimd.memset(ones128[:], 1.0)
            make_identity(nc, ident[:])

            def rope_inplace(dst, ps, sg):
                # dst (sbuf bf16 [128,512]) = rope(ps [128,512]); sg picks
                # the cos/sin column group.
                cs = cos_sb[:, sg * 512:(sg + 1) * 512]
                sn = sin_sb[:, sg * 512:(sg + 1) * 512]
                rot = rot_pool.tile([128, 512], bf, name="rot", tag="rot")
                nc.vector.tensor_copy(dst, ps)
                nc.vector.tensor_scalar_mul(rot[0:64, :], dst[64:128, :], -1.0)
                nc.vector.tensor_copy(rot[64:128, :], dst[0:64, :])
                nc.vector.tensor_mul(dst, dst, cs)
                nc.vector.tensor_mul(rot[:], rot[:], sn)
                nc.vector.tensor_add(dst, dst, rot[:])

            # ---- k/v projections, contraction-chunk outer ----
            with tc.tile_pool(name="ldps", bufs=8, space="PSUM") as ldps:
                ldk = [ldps.tile([128, 512], f32, name=f"ldk{sg}", tag="ld")
                       for sg in range(SB)]
                ldv = [ldps.tile([128, 512], f32, name=f"ldv{sg}", tag="ld")
                       for sg in range(SB)]
                for kk in range(KT):
                    for sg in range(SB):
                        nc.tensor.matmul(
                            ldv[sg][:],
                            wv_sb[:, kk * HD: kk * HD + 128],
                            hs_sb[:, kk * S + sg * 512: kk * S + sg * 512 + 512],
                            start=(kk == 0), stop=(kk == KT - 1),
                        )
                        nc.tensor.matmul(
                            ldk[sg][:],
                            wk_sb[:, kk * HD: kk * HD + 128],
                            hs_sb[:, kk * S + sg * 512: kk * S + sg * 512 + 512],
                            start=(kk == 0), stop=(kk == KT - 1),
                        )
                # v copies on ACT first so the PE transposes can start while
                # the DVE is still busy with rope(k).
                for sg in range(SB):
                    nc.scalar.copy(vT_sb[:, sg * 512:(sg + 1) * 512], ldv[sg][:])
                for sg in range(SB):
                    rope_inplace(kT_sb[:, sg * 512:(sg + 1) * 512], ldk[sg][:], sg)

            with (
                tc.tile_pool(name="tr_ps", bufs=3, space="PSUM") as tr_ps,
                tc.tile_pool(name="pv_ps", bufs=4, space="PSUM") as pv_ps,
                tc.tile_pool(name="sm_ps", bufs=1, space="PSUM") as sm_ps,
            ):
                def vtrans():
                    # ---- v back to natural [s, d] layout via PE transpose ----
                    for tj in range(ST):
                        tp = tr_ps.tile([128, 128], bf, name="tp", tag="tr")
                        nc.tensor.transpose(tp[:], vT_sb[:, tj * 128:(tj + 1) * 128], ident[:])
                        nc.vector.tensor_copy(vn_sb[:, tj * 128:(tj + 1) * 128], tp[:])

                def qproj(gi):
                    for h in range(HPC):
                        ps = tr_ps.tile([128, 512], f32, name="qps", tag="tr")
                        for kk in range(KT):
                            nc.tensor.matmul(
                                ps[:],
                                wq_sb[:, kk * QDIM + h * 128: kk * QDIM + (h + 1) * 128],
                                hs_sb[:, kk * S + gi * 512: kk * S + gi * 512 + 512],
                                start=(kk == 0), stop=(kk == KT - 1),
                            )
                        rope_inplace(
                            qT_sb[:, h * S + gi * 512: h * S + gi * 512 + 512],
                            ps[:], gi,
                        )

                def attn(gi):
                    # Per-head tiles: off-diagonal j-tiles full width, the 4
                    # diagonal j-tiles width-trimmed to the causal triangle.
                    # tiles: list of (tj, i_off, width, mask_p)
                    tiles = []
                    for tj in range(4 * gi):
                        tiles.append((tj, 0, 512, None))
                    for p in range(4):
                        tiles.append((4 * gi + p, 128 * p, 512 - 128 * p, p))
                    ntile = len(tiles)

                    smp = sm_ps.tile([128, 512], f32, name="smp", tag="sm")
                    pvs = []

                    def emit_head(h):
                        qh = qT_sb[:, h * S + gi * 512: h * S + gi * 512 + 512]
                        pv = pv_ps.tile([128, 512], f32, name="pv", tag="pv")
                        pvs.append(pv)
                        # software pipeline with one-tile lookahead: emit
                        # score(t+1) before sum/pv(t) so exp overlaps PE.
                        pbs = [None] * ntile

                        def emit_score(t):
                            tj, ioff, w, p = tiles[t]
                            sc = tr_ps.tile([128, 512], f32, name="sc", tag="tr")
                            nc.tensor.matmul(
                                sc[:, 0:w],
                                kT_sb[:, tj * 128:(tj + 1) * 128],
                                qh[:, ioff:512],
                                start=True, stop=True,
                            )
                            pb = probs_pool.tile([128, 512], bf, name="pb")
                            nc.scalar.activation(
                                pb[:, 0:w], sc[:, 0:w], Exp, scale=inv_sqrt_d
                            )
                            if p is not None:
                                nc.vector.tensor_mul(
                                    pb[:, 0:w], pb[:, 0:w],
                                    mask_sb[:, p * 512 + 128 * p:(p + 1) * 512],
                                )
                            pbs[t] = pb

                        def emit_sumpv(t):
                            tj, ioff, w, p = tiles[t]
                            nc.tensor.matmul(
                                smp[32 * h:32 * h + 1, ioff:512],
                                ones_sb[:], pbs[t][:, 0:w],
                                start=(t == 0), stop=(t == ntile - 1),
                                skip_group_check=True,
                                tile_position=(0, 32 * h),
                            )
                            nc.tensor.matmul(
                                pv[:, ioff:512],
                                vn_sb[:, tj * 128:(tj + 1) * 128], pbs[t][:, 0:w],
                                start=(t == 0), stop=(t == ntile - 1),
                                skip_group_check=True,
                            )

                        emit_score(0)
                        if ntile > 1:
                            emit_score(1)
                        for t in range(ntile):
                            if t + 2 < ntile:
                                emit_score(t + 2)
                            emit_sumpv(t)

                    for h in range(HPC):
                        emit_head(h)
                    # batched 1/sums for all 4 heads (rows 32h of smp) as
                    # exp(-ln(s)) on ACT: ln/exp/copy share one act table, and
                    # this is ~2.6us cheaper than the DVE reciprocal.  The
                    # per-head broadcast+normalize follows in fin(gi), which
                    # the caller schedules behind PE filler work.
                    lns = rcp_pool.tile([128, 512], f32, name="lns", tag="rcp")
                    nc.scalar.activation(lns[:], smp[:], Ln)
                    rcpb = rcp_pool.tile([128, 512], bf, name="rcpb", tag="rcpb")
                    nc.scalar.activation(rcpb[:], lns[:], Exp, scale=-1.0)
                    return pvs, rcpb

                def fin(gi, pvs, rcpb):
                    for h in range(HPC):
                        bc = tr_ps.tile([128, 512], f32, name="bc", tag="tr")
                        nc.tensor.matmul(
                            bc[:], ones128[32 * h:32 * h + 1, :],
                            rcpb[32 * h:32 * h + 1, :],
                            start=True, stop=True,
                            tile_position=(32 * h, 0),
                        )
                        bcs = rcp_pool.tile([128, 512], bf, name="bcs", tag="bcs")
                        nc.scalar.copy(bcs[:], bc[:])
                        a_sl = at_sb[:, h * S + gi * 512: h * S + gi * 512 + 512]
                        nc.vector.scalar_tensor_tensor(
                            a_sl, pvs[h][:], 1.0, bcs[:], Mult, Mult
                        )

                def oproj(gi):
                    for st in range(4 * gi, 4 * gi + 4):
                        ostage = ostage_pool.tile([128, HID], bf, name="ostage")
                        for eg in range(SB):
                            ps = tr_ps.tile([128, 512], f32, name="ops", tag="tr")
                            for h in range(HPC):
                                nc.tensor.matmul(
                                    ps[:],
                                    at_sb[:, h * S + st * 128: h * S + st * 128 + 128],
                                    wo_sb[:, h * HID + eg * 512: h * HID + eg * 512 + 512],
                                    start=(h == 0), stop=(h == HPC - 1),
                                )
                            o_sl = ostage[:, eg * 512:(eg + 1) * 512]
                            if eg % 2 == 0:
                                nc.vector.tensor_copy(o_sl, ps[:])
                            else:
                                nc.scalar.copy(o_sl, ps[:])
                        nc.sync.dma_start(
                            out[st * 128:(st + 1) * 128, :], ostage[:]
                        )

                # Schedule: PE filler (qproj / delayed oproj) sits between
                # attn(gi) and fin(gi) so the batched reciprocal latency is
                # hidden; oproj(gi) runs one gi late for the same reason.
                qproj(0)
                vtrans()
                qproj(1)
                f0 = attn(0)
                qproj(2)
                fin(0, *f0)
                f1 = attn(1)
                qproj(3)
                fin(1, *f1)
                f2 = attn(2)
                oproj(0)
                fin(2, *f2)
                f3 = attn(3)
                oproj(1)
                fin(3, *f3)
                oproj(2)
                oproj(3)
    _legalize_waits(nc)
    return nc


def _host_prep(hidden_states, Wq, Wk, Wv, Wo, position_ids):
    bf = ml_dtypes.bfloat16
    inv_freq = 1.0 / (THETA ** (np.arange(0, HD, 2, dtype=np.float64) / HD))

    mask = np.zeros((4, 128, 512), dtype=bf)
    jl = np.arange(128)[:, None]
    il = np.arange(512)[None, :]
    for p in range(4):
        mask[p] = (128 * p + jl <= il).astype(bf)
    # flat [128, 4*512] layout matching mask_sb
    mask_flat = np.ascontiguousarray(mask.transpose(1, 0, 2).reshape(128, 4 * 512))

    def flat(w):  # [KT,128,N] chunked -> [128, KT*N] sbuf layout
        return np.ascontiguousarray(w.transpose(1, 0, 2).reshape(128, -1))

    in_maps = []
    for d in range(NCORES):
        b, g = d // 4, d % 4
        hsT = np.ascontiguousarray(hidden_states[b].T).astype(bf).reshape(KT, 128, S)
        wqT = np.ascontiguousarray(Wq[g * QDIM:(g + 1) * QDIM].T).astype(bf).reshape(KT, 128, QDIM)
        wkT = np.ascontiguousarray(Wk[g * HD:(g + 1) * HD].T).astype(bf).reshape(KT, 128, HD)
        wvT = np.ascontiguousarray(Wv[g * HD:(g + 1) * HD].T).astype(bf).reshape(KT, 128, HD)
        woT = np.ascontiguousarray(Wo[:, g * QDIM:(g + 1) * QDIM].T).astype(bf).reshape(4, 128, HID)
        freqs = position_ids[b].astype(np.float64)[:, None] * inv_freq[None, :]  # [S, 64]
        emb = np.concatenate([freqs, freqs], axis=1)  # [S, 128]
        cosT = np.cos(emb).T.astype(bf)
        sinT = np.sin(emb).T.astype(bf)
        in_maps.append({
            "hsT": hsT, "wqT": flat(wqT), "wkT": flat(wkT), "wvT": flat(wvT),
            "woT": flat(woT),
            "cosT": np.ascontiguousarray(cosT),
            "sinT": np.ascontiguousarray(sinT),
            "mask": mask_flat,
        })
    return in_maps


def kernel(hidden_states, Wq, Wk, Wv, Wo, position_ids, _trace=False, _tmpdir=None):
    from concourse.bass_utils import run_bass_kernel_spmd

    if "nc" not in _CACHE:
        _CACHE["nc"] = _build_nc()
    nc = _CACHE["nc"]

    in_maps = _host_prep(
        np.asarray(hidden_states), np.asarray(Wq), np.asarray(Wk),
        np.asarray(Wv), np.asarray(Wo), np.asarray(position_ids),
    )
    res = run_bass_kernel_spmd(
        nc, in_maps, core_ids=list(range(NCORES)), trace=_trace, tmpdir=_tmpdir
    )
    _CACHE["last_result"] = res

    out = np.zeros((B, S, NH * HD), dtype=np.float32)
    for d in range(NCORES):
        out[d // 4] += res.results[d]["out"].astype(np.float32)
    return out
